# revision 28
# baseline (speedup 1.0000x reference)
"""Trainium2 Bass kernel for nn_AutocorrelationCorrelogram.

For nervegram [B=4, F=50, T=20000, C=2]: 300 periodic-Hann-windowed frames
of length 512 per (b,f,c) signal, circular autocorrelation via
Wiener-Khinchin (rfft -> |.|^2 -> irfft), relu, normalize by sqrt(zero
lag), keep 256 lags, mean over channels -> [4, 50, 300, 256].

Sharding: pure data parallel over the 200 (b,f) pairs -> 25 per core x 8
cores (SPMD, no collectives).

v4 design ("host-framed radix-4, lag-major irfft"):
  - Host prep (free): windowed frames, radix-4 DIT combos
    (G0 = x0+x1+x2+x3, G2 = x0-x1+x2-x3, d = x0-x2, e = x1-x3 with
    xa[b] = wx[128a+b]), shipped time-major bf16 as
    g[sb, p, c, comp, col], col = 20 frames x 25 bf = 500.
  - rfft: 6 matmuls per (c,sb) with residue-class (k mod 4)
    stationaries; output rows pack [Re | Im] per residue in 4 PSUM
    tiles [128, 500].
  - squares: X^2 via ACT Square for 2 residues; via DVE copy (PSUM->
    SBUF bf16) + Pool self-mul for the other 2 (spreads the PSUM-drain
    over all three elementwise engines). Both channels land in one
    sq[j] tile [128, c, 500].
  - irfft SWAPPED: stationary = constant Dext half [128 bins, 128
    lags], moving = sq[j] [128, 1000] (both channels) -> acf^T
    [128 lags, 1000] per half, 8 matmuls/sb (vs 32 row-major), halving
    LDWEIGHTS pressure. Dext rows repeat the bin coefficient for the
    Re/Im rows of the same bin; alpha=0.25 folds the channel mean.
  - norm is per-COLUMN now: ACT Sqrt(lag0 row + eps) [1,1000], DVE
    reciprocal, Pool partition_broadcast -> rccb [128,1000]; drains:
    ch0 via ACT Relu then Pool scale-mul, ch1 via DVE fused
    (max 0) * rccb; channel mean add on Pool; one output DMA per sb
    (sync engine) of mt [128 lag, 2 half, 500] bf16.
"""

import sys

import numpy as np

sys.path.insert(0, "/opt/trn_rl_repo")

B, F, T, C = 4, 50, 20000, 2
NUM_FRAME = 300
LEN_FRAME = 512
LAGS = 256
N_CORES = 8
BF_PER_CORE = (B * F) // N_CORES  # 25

FRAMES_PER_SB = 20
N_SB = NUM_FRAME // FRAMES_PER_SB  # 15
NCOLS = FRAMES_PER_SB * BF_PER_CORE  # 500

STARTS = np.linspace(0, T - LEN_FRAME, NUM_FRAME).astype(np.int64)


def build_weights():
    """Radix-4 rfft stationaries (6 x [128,128]) + extended irfft Dext
    (4 x [128,256], alpha folded)."""
    b = np.arange(128)

    def ang(c, kap):
        return 2.0 * np.pi * np.outer(b, 4 * kap + c) / LEN_FRAME

    k65 = np.arange(65)
    k64 = np.arange(64)
    th0 = ang(0, k65)
    stat0 = np.concatenate([np.cos(th0), -np.sin(th0[:, 1:64])], axis=1)
    th2 = ang(2, k64)
    stat2 = np.concatenate([np.cos(th2), -np.sin(th2)], axis=1)
    th1 = ang(1, k64)
    C1, S1 = np.cos(th1), np.sin(th1)
    statA = np.concatenate([C1, -S1], axis=1)  # moving d
    statB = np.concatenate([-S1, -C1], axis=1)  # moving e
    th3 = ang(3, k64)
    C3, S3 = np.cos(th3), np.sin(th3)
    statC = np.concatenate([C3, -S3], axis=1)  # moving d
    statD = np.concatenate([S3, C3], axis=1)  # moving e
    stats = np.stack([stat0, stat2, statA, statB, statC, statD])  # [6,128,128]

    alpha = 0.25  # folds channel-mean 0.5 (output scales with sqrt(alpha))
    l = np.arange(LAGS)

    def dext(bins):
        coef = np.where((bins == 0) | (bins == 256), 1.0, 2.0)
        return (alpha * coef[:, None] / LEN_FRAME) * np.cos(
            2.0 * np.pi * np.outer(bins, l) / LEN_FRAME
        )

    bins0 = np.concatenate([4 * np.arange(65), 4 * np.arange(1, 64)])
    bins2 = np.concatenate([4 * k64 + 2, 4 * k64 + 2])
    bins1 = np.concatenate([4 * k64 + 1, 4 * k64 + 1])
    bins3 = np.concatenate([4 * k64 + 3, 4 * k64 + 3])
    dexts = np.stack([dext(bins0), dext(bins2), dext(bins1), dext(bins3)])
    return stats.astype(np.float32), dexts.astype(np.float32)


def build_nc(n_sb=N_SB):
    from contextlib import ExitStack

    import concourse.bacc as bacc
    import concourse.bass as bass
    import concourse.tile as tile
    from concourse import mybir

    f32 = mybir.dt.float32
    bf16 = mybir.dt.bfloat16
    AF = mybir.ActivationFunctionType
    ALU = mybir.AluOpType

    nc = bacc.Bacc("TRN2", target_bir_lowering=False, debug=False)

    g_d = nc.dram_tensor(
        "g", [n_sb, 128, C, 4, NCOLS], bf16, kind="ExternalInput"
    ).ap()
    stats_d = nc.dram_tensor("stats", [6, 128, 128], bf16, kind="ExternalInput").ap()
    dext_d = nc.dram_tensor("dext", [4, 128, LAGS], bf16, kind="ExternalInput").ap()
    # lag-major out: [sb, half, lag128, col(mm,bf)]; host un-permutes.
    out = nc.dram_tensor(
        "out", [n_sb, 2, 128, NCOLS], bf16, kind="ExternalOutput"
    ).ap()

    with tile.TileContext(nc) as tc, ExitStack() as ctx:
        consts = ctx.enter_context(tc.tile_pool(name="consts", bufs=1))
        work = ctx.enter_context(tc.tile_pool(name="work", bufs=1))
        pp = ctx.enter_context(tc.tile_pool(name="ps", bufs=1, space="PSUM"))

        # ---- constants ----
        stats_sb = consts.tile([128, 6, 128], bf16, tag="stats")
        for j in range(6):
            nc.sync.dma_start(out=stats_sb[:, j, :], in_=stats_d[j])
        dext_sb = consts.tile([128, 4, LAGS], bf16, tag="dext")
        for j in range(4):
            nc.sync.dma_start(out=dext_sb[:, j, :], in_=dext_d[j])
        zero_b = consts.tile([128, 1], f32, tag="zerob")
        nc.vector.memset(zero_b[:], 0.0)
        eps_b = consts.tile([128, 1], f32, tag="epsb")
        nc.vector.memset(eps_b[:], 1e-30)
        ones_b = consts.tile([1, 128], f32, tag="onesb")
        nc.vector.memset(ones_b[:], 1.0)

        def load_sb(s):
            gt = work.tile([128, C, 4, NCOLS], bf16, tag="gt", bufs=4)
            nc.sync.dma_start(out=gt[:], in_=g_d[s])
            return gt

        gt_queue = {}
        for s in range(min(2, n_sb)):
            gt_queue[s] = load_sb(s)

        for sb in range(n_sb):
            if sb + 2 < n_sb:
                gt_queue[sb + 2] = load_sb(sb + 2)
            gt = gt_queue.pop(sb)

            # sq[j]: [128, c, 500] bf16, both channels
            sqs = [
                work.tile([128, C, NCOLS], bf16, tag=f"sq{j}", bufs=2, name=f"sq{j}")
                for j in range(4)
            ]
            for c in range(C):
                # ---- radix-4 rfft: 6 matmuls -> 4 PSUM tiles ----
                ps = [
                    pp.tile(
                        [128, NCOLS], f32, tag=f"rf{j}", bufs=1, name=f"rf{j}"
                    )
                    for j in range(4)
                ]
                nc.tensor.matmul(
                    ps[0][:], stats_sb[:, 0, :], gt[:, c, 0, :],
                    start=True, stop=True,
                )
                nc.tensor.matmul(
                    ps[1][:], stats_sb[:, 1, :], gt[:, c, 1, :],
                    start=True, stop=True,
                )
                nc.tensor.matmul(
                    ps[2][:], stats_sb[:, 2, :], gt[:, c, 2, :],
                    start=True, stop=False,
                )
                nc.tensor.matmul(
                    ps[2][:], stats_sb[:, 3, :], gt[:, c, 3, :],
                    start=False, stop=True,
                )
                nc.tensor.matmul(
                    ps[3][:], stats_sb[:, 4, :], gt[:, c, 2, :],
                    start=True, stop=False,
                )
                nc.tensor.matmul(
                    ps[3][:], stats_sb[:, 5, :], gt[:, c, 3, :],
                    start=False, stop=True,
                )

                # ---- squares: 3 on ACT, 1 via DVE cast + Pool self-mul ----
                for j in range(4):
                    if j < 3:
                        nc.scalar.activation(
                            sqs[j][:, c, :], ps[j][:], AF.Square, bias=zero_b[:]
                        )
                    else:
                        cp = work.tile(
                            [128, NCOLS], bf16, tag="cp", bufs=4, name="cp"
                        )
                        nc.vector.tensor_copy(cp[:], ps[j][:])
                        nc.gpsimd.tensor_mul(sqs[j][:, c, :], cp[:], cp[:])

            # ---- swapped irfft: acf^T [128 lag, 500] per (c, half) ----
            # (matmul out must fit one PSUM bank: <= 512 f32 cols)
            acfts = {}
            for c in range(C):
                for h in range(2):
                    acft = pp.tile([128, NCOLS], f32, tag="acfT", bufs=4)
                    for j in range(4):
                        nc.tensor.matmul(
                            acft[:],
                            dext_sb[:, j, 128 * h : 128 * h + 128],
                            sqs[j][:, c, :],
                            start=(j == 0), stop=(j == 3),
                        )
                    acfts[c, h] = acft

            # ---- per-column norm: ACT sqrt of lag-0 row -> SBUF->SBUF DMA
            # broadcast (stride-0 partition source, on idle Pool) -> DVE
            # fast reciprocal full-width ----
            rccbs = []
            for c in range(C):
                sqc = work.tile([1, NCOLS], f32r, tag="sqc", bufs=4)
                nc.scalar.activation(
                    sqc[:], acfts[c, 0][0:1, :], AF.Sqrt, bias=eps_b[0:1]
                )
                sqcb = pp.tile(
                    [128, NCOLS], f32, tag=f"rf{2 + c}", bufs=1,
                    name=f"sqcb{c}",
                )
                nc.tensor.matmul(
                    sqcb[:], ones_b[:].bitcast(f32r), sqc[:],
                    start=True, stop=True,
                )
                rccb = work.tile([128, NCOLS], f32, tag="rccb", bufs=4)
                nc.vector.reciprocal_approx_fast(out=rccb[:], in_=sqcb[:])
                rccbs.append(rccb)

            # drains: DVE fused (max 0) * rccb for both channels -> halves
            # of one tile each, then a single Pool add for the channel mean
            nt0 = work.tile([128, 2, NCOLS], bf16, tag="nt0", bufs=3)
            nt1 = work.tile([128, 2, NCOLS], bf16, tag="nt1", bufs=3)
            for h in range(2):
                nc.vector.scalar_tensor_tensor(
                    nt0[:, h, :], acfts[0, h][:], 0.0,
                    rccbs[0][:], ALU.max, ALU.mult,
                )
                nc.vector.scalar_tensor_tensor(
                    nt1[:, h, :], acfts[1, h][:], 0.0,
                    rccbs[1][:], ALU.max, ALU.mult,
                )
            mt = work.tile([128, 2, NCOLS], bf16, tag="mt", bufs=3)
            nc.gpsimd.tensor_add(mt[:], nt0[:], nt1[:])

            nc.sync.dma_start(
                out=out[sb].rearrange("h l n -> l h n"), in_=mt[:]
            )

    nc.compile()
    return nc


_NC_CACHE = {}


def _get_nc():
    if "nc" not in _NC_CACHE:
        _NC_CACHE["nc"] = build_nc()
    return _NC_CACHE["nc"]


def host_prep(nerv):
    """Per-core inputs: windowed frames, radix-4 combos, time-major bf16."""
    import ml_dtypes

    t = np.arange(LEN_FRAME, dtype=np.float64)
    window = (0.5 - 0.5 * np.cos(2.0 * np.pi * t / LEN_FRAME)).astype(np.float32)
    stats, dexts = build_weights()
    stats_bf = stats.astype(ml_dtypes.bfloat16)
    dext_bf = dexts.astype(ml_dtypes.bfloat16)

    xs = nerv.reshape(B * F, T, C)
    idx = STARTS[:, None] + np.arange(LEN_FRAME)  # [300, 512]
    in_maps = []
    for i in range(N_CORES):
        sl = xs[BF_PER_CORE * i : BF_PER_CORE * (i + 1)]  # [25, T, 2]
        sig = np.ascontiguousarray(sl.transpose(2, 0, 1))  # [2, 25, T]
        frames = sig[:, :, idx]  # [2, 25, 300, 512]
        wx = frames * window
        wxa = wx.reshape(C, BF_PER_CORE, NUM_FRAME, 4, 128)
        x0, x1, x2, x3 = (wxa[..., a, :] for a in range(4))
        u = x0 + x2
        v = x1 + x3
        comb = np.stack([u + v, u - v, x0 - x2, x1 - x3])  # [4comp,2,25,300,128]
        # -> [sb, p(128), c, comp, mm, bf]
        comb = comb.reshape(4, C, BF_PER_CORE, N_SB, FRAMES_PER_SB, 128)
        g = np.ascontiguousarray(
            comb.transpose(3, 5, 1, 0, 4, 2)
        ).reshape(N_SB, 128, C, 4, NCOLS)
        in_maps.append(
            {
                "g": g.astype(ml_dtypes.bfloat16),
                "stats": stats_bf,
                "dext": dext_bf,
            }
        )
    return in_maps


def kernel(nervegram, trace=False, use_f32r=True, bf16_front=False):
    from concourse.bass_utils import run_bass_kernel_spmd

    nerv = np.ascontiguousarray(np.asarray(nervegram, dtype=np.float32))
    assert nerv.shape == (B, F, T, C)
    in_maps = host_prep(nerv)
    nc = _get_nc()
    res = run_bass_kernel_spmd(nc, in_maps, list(range(N_CORES)), trace=trace)
    # per-core out [sb, h, lag128, col] with col = 25*mm + bf;
    # frame = 20*sb + mm, lag = 128*h + lag128
    cores = [
        np.ascontiguousarray(
            res.results[i]["out"]
            .astype(np.float32)
            .reshape(N_SB, 2, 128, FRAMES_PER_SB, BF_PER_CORE)
            .transpose(4, 0, 3, 1, 2)
        ).reshape(BF_PER_CORE, NUM_FRAME, LAGS)
        for i in range(N_CORES)
    ]
    out = np.concatenate(cores, axis=0).reshape(B, F, NUM_FRAME, LAGS)
    if trace:
        return out, res
    return out
